# revision 18
# baseline (speedup 1.0000x reference)
"""CPSF fused codebook kernel for Trainium2, M-sharded across 8 NeuronCores.

Math (closed form of the reference's K=8 Gauss-Hermite quadrature, which is
exactly the Gaussian integral it approximates; agreement ~4e-5 relative):

    rho[b,m]  = (scale_m/sqrt(2)) * exp(-pi*(pr^2/(2*sp) + pi^2/sp + perp2/sq))
    w[b,m]    = alpha_m * |<dj_hat, d_hat>|^2 * rho
    out[b,s]  = (w @ T) / (sum_m w + eps)

All normalizations are factored into per-m / per-b scalars:
  - dj norm:  pr = pr_u/|dj|, align = align_u/(|dj|^2 |d|^2)  ->  per-m
    constants c1=-pi/(2*sp*|dj|^2), c2=2*c1, c3=-pi/sq, and a log-space
    gain logg = ln(alpha) + ln(sp)/2 - ln(2*pi)/2 - 2*ln|dj|^2 applied as
    the exp() bias.
  - d norm: the per-query factor 1/|d_b|^2 cancels in num/den; it only
    survives in eps, so each core adds (eps/8)*|d_b|^2 to its partial
    denominator before the cross-core reduction.

Sharding: codebook axis M=8192 split 8 ways (M_loc=1024).  Each core
computes partial num[b, 512] = w' @ [T_re|T_im] and partial den[b], then a
ReduceScatter hands core c the b-rows [128c, 128c+128) of num and the
matching den slice; the core divides and writes its own [128, 512] chunk.
"""

import math
import numpy as np

import concourse.bacc as bacc
import concourse.tile as tile
import concourse.mybir as mybir
from concourse.bass_utils import run_bass_kernel_spmd

F32 = mybir.dt.float32
F32R = mybir.dt.float32r
AF = mybir.ActivationFunctionType
OP = mybir.AluOpType

B, N, M, S = 1024, 128, 8192, 256
NCORES = 8
MLOC = M // NCORES          # 1024 codes per core
MT = MLOC // 128            # 8 m-tiles per core
BH = 2                      # two 512-wide b halves
BF = B // BH                # 512
BC = B // 128               # 8 b chunks of 128
EPS_TOTAL = 1e-3
PI = math.pi


DEBUG_OUTPUTS = False


def _build():
    nc = bacc.Bacc("TRN2", target_bir_lowering=False, debug=False,
                   num_devices=NCORES)
    dbg = {}

    def dbg_dump(name, ap):
        if not DEBUG_OUTPUTS:
            return
        t = nc.dram_tensor("dbg_" + name, list(ap.shape), F32,
                           kind="ExternalOutput").ap()
        nc.sync.dma_start(t, ap)
        dbg[name] = ap.shape

    def din(name, shape, dt=F32R):
        return nc.dram_tensor(name, shape, dt, kind="ExternalInput").ap()

    zrT = din("zrT", [128, B])          # z_re transposed [n, b]
    ziT = din("ziT", [128, B])
    drT = din("drT", [128, B])          # d (unnormalized) transposed
    diT = din("diT", [128, B])
    zjrT = din("zjrT", [128, MLOC])     # codebook slice, transposed [n, m]
    zjiT = din("zjiT", [128, MLOC])
    djrT = din("djrT", [128, MLOC])
    djiT = din("djiT", [128, MLOC])
    Tcat = din("Tcat", [128, MT * 512])  # [T_re|T_im] m-chunked: col mc*512
    alpha_d = din("alpha", [128, MT], F32)   # per-m vectors, [m%128, m//128]
    sp_d = din("sigp", [128, MT], F32)
    sq_d = din("sigq", [128, MT], F32)

    out_d = nc.dram_tensor("out", [128, 512], F32, kind="ExternalOutput").ap()

    with tile.TileContext(nc) as tc:
        with tc.tile_pool(name="inp", bufs=1) as inp, \
             tc.tile_pool(name="dram", bufs=1, space="DRAM") as dram:
            # ---- persistent SBUF inputs ----
            zr = inp.tile([128, B], F32R, tag="zr")
            zi = inp.tile([128, B], F32R, tag="zi")
            dr = inp.tile([128, B], F32R, tag="dr")
            di = inp.tile([128, B], F32R, tag="di")
            zjr = inp.tile([128, MLOC], F32R, tag="zjr")
            zji = inp.tile([128, MLOC], F32R, tag="zji")
            djr = inp.tile([128, MLOC], F32R, tag="djr")
            dji = inp.tile([128, MLOC], F32R, tag="dji")
            tt = inp.tile([128, MT * 512], F32R, tag="tt")
            al_sb = inp.tile([128, MT], F32, tag="al")
            sp_sb = inp.tile([128, MT], F32, tag="sp")
            sq_sb = inp.tile([128, MT], F32, tag="sq")
            for t, d in [(zr, zrT), (zi, ziT), (dr, drT), (di, diT),
                         (zjr, zjrT), (zji, zjiT), (djr, djrT), (dji, djiT),
                         (tt, Tcat), (al_sb, alpha_d), (sp_sb, sp_d),
                         (sq_sb, sq_d)]:
                nc.sync.dma_start(t[:], d)

            zr_v, zi_v = zr[:].bitcast(F32), zi[:].bitcast(F32)
            dr_v, di_v = dr[:].bitcast(F32), di[:].bitcast(F32)
            zjr_v, zji_v = zjr[:].bitcast(F32), zji[:].bitcast(F32)
            djr_v, dji_v = djr[:].bitcast(F32), dji[:].bitcast(F32)

            # derived full-width operands
            m2zjr = inp.tile([128, MLOC], F32R, tag="m2zjr")
            m2zji = inp.tile([128, MLOC], F32R, tag="m2zji")
            nji = inp.tile([128, MLOC], F32R, tag="nji")
            nc.vector.tensor_scalar_mul(m2zjr[:], zjr_v, -2.0)
            nc.vector.tensor_scalar_mul(m2zji[:], zji_v, -2.0)
            nc.vector.tensor_scalar_mul(nji[:], dji_v, -1.0)

            # ones / -ones columns and ones row (f32r for matmul use)
            onef = inp.tile([128, 1], F32, tag="onef")
            ones_c = inp.tile([128, 1], F32R, tag="ones_c")
            mones_c = inp.tile([128, 1], F32R, tag="mones_c")
            ones_r = inp.tile([1, 128], F32R, tag="ones_r")
            onef_r = inp.tile([1, 128], F32, tag="onef_r")
            ones_rB = inp.tile([1, BF], F32R, tag="ones_rB")
            onef_rB = inp.tile([1, BF], F32, tag="onef_rB")
            monef = inp.tile([128, 1], F32, tag="monef")
            nc.vector.memset(onef[:], 1.0)
            nc.vector.memset(monef[:], -1.0)
            nc.vector.tensor_copy(ones_c[:], onef[:])
            nc.vector.tensor_scalar_mul(mones_c[:], onef[:], -1.0)
            nc.vector.memset(onef_r[:], 1.0)
            nc.vector.tensor_copy(ones_r[:], onef_r[:])
            nc.vector.memset(onef_rB[:], 1.0)
            nc.vector.tensor_copy(ones_rB[:], onef_rB[:])

            # ---- preamble reductions over the partition (n) axis ----
            zn2row = inp.tile([1, B], F32R, tag="zn2row")
            zjn2row = inp.tile([1, MLOC], F32R, tag="zjn2row")
            dn2row = inp.tile([1, B], F32, tag="dn2row")
            djn2_sb = inp.tile([128, MT], F32, tag="djn2")
            mcpr_sb = inp.tile([128, MT], F32, tag="mcpr")
            mcpi_sb = inp.tile([128, MT], F32, tag="mcpi")

            with tc.tile_pool(name="prew", bufs=2) as prew, \
                 tc.tile_pool(name="prer", bufs=2) as prer, \
                 tc.tile_pool(name="preps", bufs=2, space="PSUM") as preps, \
                 tc.tile_pool(name="pmps", bufs=2, space="PSUM") as pmps:

                def products(pa, pb, sub=False):
                    """per-element pa[0]*pa[1] (+/-) pb[0]*pb[1] -> f32r tile"""
                    w = pa[0].shape[1]
                    u1 = prew.tile([128, w], F32, tag="u1", name="u1")
                    u2 = prew.tile([128, w], F32, tag="u2", name="u2")
                    u3 = prer.tile([128, w], F32R, tag="u3", name="u3")
                    nc.vector.tensor_mul(u1[:], pa[0], pa[1])
                    nc.vector.tensor_mul(u2[:], pb[0], pb[1])
                    nc.vector.tensor_tensor(u3[:], u1[:], u2[:],
                                            OP.subtract if sub else OP.add)
                    return u3

                def rowreduce(u3, dst):
                    """sum over partitions -> [1, W] row (via ones matmul)"""
                    w = dst[:].shape[1]
                    ps = preps.tile([1, w], F32, tag="ps", name="ps")
                    for h in range(w // 512):
                        sl = slice(h * 512, (h + 1) * 512)
                        nc.tensor.matmul(ps[0:1, sl], ones_c[:], u3[:, sl],
                                         start=True, stop=True)
                    nc.scalar.copy(dst[:], ps[:])

                def pmreduce(u3, dst, ones):
                    """sum over partitions -> per-partition [128, MT] layout
                    via per-m-tile matmul transpose (out[m,1] = u3_mt.T @ 1).
                    Plain fp32 (free dim 1 violates fp32r ISA restrictions)."""
                    for mt in range(MT):
                        msl = slice(mt * 128, (mt + 1) * 128)
                        ps1 = pmps.tile([128, 1], F32, tag="ps1", name="ps1")
                        nc.tensor.matmul(ps1[:], u3[:, msl].bitcast(F32),
                                         ones[:], start=True, stop=True)
                        nc.scalar.copy(dst[:, mt:mt + 1], ps1[:])

                u = products((zr_v, zr_v), (zi_v, zi_v))
                rowreduce(u, zn2row)
                u = products((zjr_v, zjr_v), (zji_v, zji_v))
                rowreduce(u, zjn2row)
                u = products((dr_v, dr_v), (di_v, di_v))
                rowreduce(u, dn2row)
                u = products((djr_v, djr_v), (dji_v, dji_v))
                pmreduce(u, djn2_sb, onef)
                u = products((djr_v, zjr_v), (dji_v, zji_v))
                pmreduce(u, mcpr_sb, monef)            # -> -cpr
                u = products((djr_v, zji_v), (dji_v, zjr_v), sub=True)
                pmreduce(u, mcpi_sb, monef)            # -> -cpi

            # ---- per-m scalar constants [128, MT] ----
            c1_sb = inp.tile([128, MT], F32, tag="c1")
            c2_sb = inp.tile([128, MT], F32, tag="c2")
            c3_sb = inp.tile([128, MT], F32, tag="c3")
            minv_sb = inp.tile([128, MT], F32, tag="minv")
            logg_sb = inp.tile([128, MT], F32, tag="logg")
            t_a = inp.tile([128, MT], F32, tag="t_a")
            t_b = inp.tile([128, MT], F32, tag="t_b")
            t_c = inp.tile([128, MT], F32, tag="t_c")
            nc.vector.tensor_mul(t_a[:], sp_sb[:], djn2_sb[:])
            nc.vector.reciprocal(t_b[:], t_a[:])
            nc.vector.tensor_scalar_mul(c1_sb[:], t_b[:], -PI / 2.0)
            nc.vector.tensor_scalar_mul(c2_sb[:], t_b[:], -PI)
            nc.vector.reciprocal(t_c[:], sq_sb[:])
            nc.vector.tensor_scalar_mul(c3_sb[:], t_c[:], -PI)
            nc.vector.reciprocal(minv_sb[:], djn2_sb[:])
            nc.vector.tensor_scalar_mul(minv_sb[:], minv_sb[:], -1.0)
            la = inp.tile([128, MT], F32, tag="la")
            lsp = inp.tile([128, MT], F32, tag="lsp")
            ldj = inp.tile([128, MT], F32, tag="ldj")
            nc.scalar.activation(la[:], al_sb[:], AF.Ln)
            nc.scalar.activation(lsp[:], sp_sb[:], AF.Ln)
            nc.scalar.activation(ldj[:], djn2_sb[:], AF.Ln)
            nc.vector.scalar_tensor_tensor(logg_sb[:], lsp[:], 0.5, la[:],
                                           op0=OP.mult, op1=OP.add)
            nc.vector.tensor_sub(logg_sb[:], logg_sb[:], ldj[:])
            nc.vector.tensor_scalar_add(logg_sb[:], logg_sb[:],
                                        -0.5 * math.log(2.0 * PI))
            dbg_dump("zn2row", zn2row[:].bitcast(F32))
            dbg_dump("zjn2row", zjn2row[:].bitcast(F32))
            dbg_dump("dn2row", dn2row[:])
            dbg_dump("djn2", djn2_sb[:])
            dbg_dump("mcpr", mcpr_sb[:])
            dbg_dump("mcpi", mcpi_sb[:])
            dbg_dump("c1", c1_sb[:])
            dbg_dump("c3", c3_sb[:])
            dbg_dump("minv", minv_sb[:])
            dbg_dump("logg", logg_sb[:])

            # ---- main passes ----
            w_all = inp.tile([128, MT * B], F32R, tag="w_all")  # w'[m, b]
            den_acc0 = inp.tile([128, BF], F32, tag="den_acc0")
            den_acc1 = inp.tile([128, BF], F32, tag="den_acc1")
            den_accs = [den_acc0, den_acc1]
            den_cat = inp.tile([1, B], F32, tag="den_cat")

            num_bounce = dram.tile([B, 512], F32)
            den_bounce = dram.tile([1, B], F32)
            num_slice = dram.tile([128, 512], F32)
            den_slice = dram.tile([1, 128], F32)

            with tc.tile_pool(name="wk", bufs=5, space="PSUM") as wk, \
                 tc.tile_pool(name="dps", bufs=1, space="PSUM") as dps, \
                 tc.tile_pool(name="ops", bufs=2, space="PSUM") as ops, \
                 tc.tile_pool(name="sb1", bufs=2) as sb1:

                def pass2(bc_list):
                    for bc in bc_list:
                        o_ps = ops.tile([128, 512], F32, tag="o_ps")
                        for mc in range(MT):
                            lhs = w_all[:, mc * B + bc * 128:
                                        mc * B + bc * 128 + 128]
                            rhs = tt[:, mc * 512:(mc + 1) * 512]
                            nc.tensor.matmul(o_ps[:], lhs, rhs,
                                             start=(mc == 0), stop=(mc == MT - 1))
                        o_cp = sb1.tile([128, 512], F32, tag="o_cp")
                        nc.scalar.copy(o_cp[:], o_ps[:])
                        nc.sync.dma_start(
                            num_bounce[bc * 128:(bc + 1) * 128, :], o_cp[:])

                for bh in range(BH):
                    bsl = slice(bh * BF, (bh + 1) * BF)
                    dacc = den_accs[bh]
                    for mt in range(MT):
                        msl = slice(mt * 128, (mt + 1) * 128)
                        mc1 = c1_sb[:, mt:mt + 1]
                        mc2 = c2_sb[:, mt:mt + 1]
                        mc3 = c3_sb[:, mt:mt + 1]
                        mmi = minv_sb[:, mt:mt + 1]
                        mlg = logg_sb[:, mt:mt + 1]
                        mbr = mcpr_sb[:, mt:mt + 1]
                        mbi = mcpi_sb[:, mt:mt + 1]

                        pr_ps = wk.tile([128, BF], F32, tag="wk")
                        pi_ps = wk.tile([128, BF], F32, tag="wk")
                        dot_ps = wk.tile([128, BF], F32, tag="wk")
                        ar_ps = wk.tile([128, BF], F32, tag="wk")
                        ai_ps = wk.tile([128, BF], F32, tag="wk")

                        nc.tensor.matmul(pr_ps[:], djr[:, msl], zr[:, bsl],
                                         start=True, stop=False)
                        nc.tensor.matmul(pr_ps[:], dji[:, msl], zi[:, bsl],
                                         start=False, stop=True)
                        nc.tensor.matmul(pi_ps[:], djr[:, msl], zi[:, bsl],
                                         start=True, stop=False)
                        nc.tensor.matmul(pi_ps[:], nji[:, msl], zr[:, bsl],
                                         start=False, stop=True)
                        nc.tensor.matmul(dot_ps[:], m2zjr[:, msl], zr[:, bsl],
                                         start=True, stop=False)
                        nc.tensor.matmul(dot_ps[:], m2zji[:, msl], zi[:, bsl],
                                         start=False, stop=False)
                        # dz2 = zn2[b] + zjn2[m] - 2*dot via two rank-1 updates
                        nc.tensor.matmul(dot_ps[:], ones_r[:],
                                         zn2row[0:1, bsl],
                                         start=False, stop=False)
                        nc.tensor.matmul(dot_ps[:], zjn2row[0:1, msl],
                                         ones_rB[:],
                                         start=False, stop=True)
                        nc.tensor.matmul(ar_ps[:], djr[:, msl], dr[:, bsl],
                                         start=True, stop=False)
                        nc.tensor.matmul(ar_ps[:], dji[:, msl], di[:, bsl],
                                         start=False, stop=True)
                        nc.tensor.matmul(ai_ps[:], djr[:, msl], di[:, bsl],
                                         start=True, stop=False)
                        nc.tensor.matmul(ai_ps[:], nji[:, msl], dr[:, bsl],
                                         start=False, stop=True)

                        pr2 = sb1.tile([128, BF], F32, tag="pr2")
                        pi2 = sb1.tile([128, BF], F32, tag="pi2")
                        ar2 = sb1.tile([128, BF], F32, tag="ar2")
                        ai2 = sb1.tile([128, BF], F32, tag="ai2")
                        nc.scalar.activation(pr2[:], pr_ps[:], AF.Square,
                                             bias=mbr)
                        nc.scalar.activation(pi2[:], pi_ps[:], AF.Square,
                                             bias=mbi)
                        nc.scalar.activation(ar2[:], ar_ps[:], AF.Square)
                        nc.scalar.activation(ai2[:], ai_ps[:], AF.Square)

                        q = sb1.tile([128, BF], F32, tag="q")
                        s = sb1.tile([128, BF], F32, tag="s")
                        t1 = sb1.tile([128, BF], F32, tag="t1")
                        e1 = sb1.tile([128, BF], F32, tag="e1")
                        e2 = sb1.tile([128, BF], F32, tag="e2")
                        eg = sb1.tile([128, BF], F32, tag="eg")
                        align = sb1.tile([128, BF], F32, tag="align")
                        nc.vector.tensor_add(q[:], pr2[:], pi2[:])
                        # s = dz2 - q/|dj|^2
                        nc.vector.scalar_tensor_tensor(s[:], q[:], mmi,
                                                       dot_ps[:],
                                                       op0=OP.mult, op1=OP.add)
                        # t1 = min(c3*s, 0) = c3 * relu(s)   (c3 < 0)
                        nc.vector.tensor_scalar(t1[:], s[:], mc3, 0.0,
                                                op0=OP.mult, op1=OP.min)
                        nc.vector.scalar_tensor_tensor(e1[:], pi2[:], mc2,
                                                       t1[:],
                                                       op0=OP.mult, op1=OP.add)
                        nc.vector.scalar_tensor_tensor(e2[:], pr2[:], mc1,
                                                       e1[:],
                                                       op0=OP.mult, op1=OP.add)
                        nc.scalar.activation(eg[:], e2[:], AF.Exp, bias=mlg)
                        nc.vector.tensor_add(align[:], ar2[:], ai2[:])
                        wv = w_all[:, mt * B + bh * BF: mt * B + bh * BF + BF]
                        nc.vector.tensor_tensor(wv, eg[:], align[:], OP.mult)
                        if bh == 0 and mt == 0:
                            dbg_dump("pr2", pr2[:])
                            dbg_dump("pi2", pi2[:])
                            dbg_dump("ar2", ar2[:])
                            dbg_dump("q", q[:])
                            dbg_dump("s", s[:])
                            dbg_dump("t1", t1[:])
                            dbg_dump("e2", e2[:])
                            dbg_dump("eg", eg[:])
                            dbg_dump("w0", wv.bitcast(F32))
                        if mt == 0:
                            nc.vector.tensor_copy(dacc[:], wv.bitcast(F32))
                        else:
                            nc.vector.tensor_add(dacc[:], dacc[:],
                                                 wv.bitcast(F32))

                    # partial denominator for this b-half
                    dacc_r = sb1.tile([128, BF], F32R, tag="dacc_r")
                    nc.vector.tensor_copy(dacc_r[:], dacc[:])
                    den_ps = dps.tile([1, BF], F32, tag="den_ps")
                    nc.tensor.matmul(den_ps[:], ones_c[:], dacc_r[:],
                                     start=True, stop=True)
                    nc.scalar.copy(den_cat[0:1, bsl], den_ps[:])

                    pass2([0, 1, 2, 3] if bh == 0 else [4, 5, 6, 7])

                # fold eps/8 * |d_b|^2 into partial den, ship to collectives
                den_cat2 = inp.tile([1, B], F32, tag="den_cat2")
                nc.vector.scalar_tensor_tensor(den_cat2[:], dn2row[:],
                                               EPS_TOTAL / NCORES, den_cat[:],
                                               op0=OP.mult, op1=OP.add)
                nc.sync.dma_start(den_bounce[:], den_cat2[:])
                dbg_dump("den_cat2", den_cat2[:])

                nc.gpsimd.collective_compute(
                    "ReduceScatter", OP.add,
                    replica_groups=[list(range(NCORES))],
                    ins=[num_bounce.opt()], outs=[num_slice.opt()])
                nc.gpsimd.collective_compute(
                    "ReduceScatter", OP.add,
                    replica_groups=[list(range(NCORES))],
                    ins=[den_bounce.opt()], outs=[den_slice.opt()])

            # ---- tail: divide own chunk ----
            with tc.tile_pool(name="tps", bufs=1, space="PSUM") as tps, \
                 tc.tile_pool(name="tsb", bufs=1) as tsb:
                o_sb = tsb.tile([128, 512], F32, tag="o_sb")
                den_row = tsb.tile([1, 128], F32, tag="den_row")
                den_col = tsb.tile([128, 1], F32, tag="den_col")
                invden = tsb.tile([128, 1], F32, tag="invden")
                outf = tsb.tile([128, 512], F32, tag="outf")
                nc.sync.dma_start(o_sb[:], num_slice[:])
                nc.sync.dma_start(den_row[:], den_slice[:])
                # transpose [1,128] -> [128,1] on the PE (fp32 rank-1 matmul)
                den_colps = tps.tile([128, 1], F32, tag="den_colps")
                nc.tensor.matmul(den_colps[:], den_row[:],
                                 onef_r[0:1, 0:1], start=True, stop=True)
                nc.scalar.copy(den_col[:], den_colps[:])
                nc.vector.reciprocal(invden[:], den_col[:])
                nc.vector.tensor_scalar_mul(outf[:], o_sb[:], invden[:])
                nc.sync.dma_start(out_d, outf[:])

    nc.compile()
    return nc


_NC_CACHE = None


def _get_nc():
    global _NC_CACHE
    if _NC_CACHE is None:
        _NC_CACHE = _build()
    return _NC_CACHE


def _in_maps(inputs):
    f32 = np.float32
    t = lambda a: np.ascontiguousarray(np.asarray(a, f32).T)
    zrT, ziT = t(inputs["z_re"]), t(inputs["z_im"])
    drT, diT = t(inputs["d_re"]), t(inputs["d_im"])
    permv = lambda v: np.ascontiguousarray(
        np.asarray(v, f32).reshape(MT, 128).T)
    maps = []
    for c in range(NCORES):
        sl = slice(c * MLOC, (c + 1) * MLOC)
        Tc = np.concatenate([np.asarray(inputs["T_re"][sl], f32),
                             np.asarray(inputs["T_im"][sl], f32)], axis=1)
        Tc = np.ascontiguousarray(
            Tc.reshape(MT, 128, 512).transpose(1, 0, 2).reshape(128, MT * 512))
        maps.append({
            "zrT": zrT, "ziT": ziT, "drT": drT, "diT": diT,
            "zjrT": t(inputs["zj_re"][sl]), "zjiT": t(inputs["zj_im"][sl]),
            "djrT": t(inputs["dj_re"][sl]), "djiT": t(inputs["dj_im"][sl]),
            "Tcat": Tc,
            "alpha": permv(inputs["alpha"][sl]),
            "sigp": permv(inputs["sigma_par"][sl]),
            "sigq": permv(inputs["sigma_perp"][sl]),
        })
    return maps


def _gather(results):
    full = np.concatenate([results[c]["out"] for c in range(NCORES)], axis=0)
    return np.ascontiguousarray(
        np.stack([full[:, :S], full[:, S:]], axis=-1).astype(np.float32))


def run(inputs, trace=False):
    nc = _get_nc()
    res = run_bass_kernel_spmd(nc, _in_maps(inputs), list(range(NCORES)),
                               trace=trace)
    return _gather(res.results), res


def kernel(**inputs) -> np.ndarray:
    out, _ = run(inputs, trace=False)
    return out


# revision 23
# speedup vs baseline: 1.1025x; 1.1025x over previous
"""CPSF fused codebook kernel for Trainium2, M-sharded across 8 NeuronCores.

Math (closed form of the reference's K=8 Gauss-Hermite quadrature, which is
exactly the Gaussian integral it approximates; agreement ~4e-5 relative):

    rho[b,m]  = (scale_m/sqrt(2)) * exp(-pi*(pr^2/(2*sp) + pi^2/sp + perp2/sq))
    w[b,m]    = alpha_m * |<dj_hat, d_hat>|^2 * rho
    out[b,s]  = (w @ T) / (sum_m w + eps)

All normalizations fold into per-m scalars (|dj|^2) or per-b factors (|d|^2,
which cancels in num/den and survives only in eps, so each core adds
(eps/8)*|d_b|^2 to its partial denominator before the reduction).

Per (m-tile, b-half) iteration, layout [m=partition, b=free]:
  PE:   pr_u, pi_u (proj), dot (full dz2 via two rank-1 updates), ar_u, ai_u
        (alignment), all f32r GEMMs; plus the running denominator row.
  ACT:  c1p = -c1*pr_u^2 and c2p = -c2*pi_u^2 via Square(scale*x + bias)
        with per-partition scale=sqrt(-c)/|dj|-folded constants; ar2/ai2
        squares (bf16); final exp with log-gain bias (bf16).
  DVE:  s = dz2 - (pr^2+pi^2)/|dj|^2 rebuilt from c1p/c2p; clamp; e2; w.
  GPS:  c1p+c2p combine (idle engine).

Cross-core: M=8192 split 8 ways.  num = w @ [T_re|T_im] (bf16 weights) is
reduced with TWO ReduceScatters (first half issued mid-kernel to overlap
compute); den uses a b-permuted layout so its ReduceScatter hands each core
exactly the denominator entries for the num rows it received.
"""

import math
import numpy as np

import concourse.bacc as bacc
import concourse.tile as tile
import concourse.mybir as mybir
from concourse.bass_utils import run_bass_kernel_spmd

F32 = mybir.dt.float32
F32R = mybir.dt.float32r
BF16 = mybir.dt.bfloat16
AF = mybir.ActivationFunctionType
OP = mybir.AluOpType

B, N, M, S = 1024, 128, 8192, 256
NCORES = 8
MLOC = M // NCORES          # 1024 codes per core
MT = MLOC // 128            # 8 m-tiles per core
BH = 2                      # two 512-wide b halves
BF = B // BH                # 512
EPS_TOTAL = 1e-3
PI = math.pi

DEBUG_OUTPUTS = False


def _build():
    nc = bacc.Bacc("TRN2", target_bir_lowering=False, debug=False,
                   num_devices=NCORES)

    def dbg_dump(name, ap):
        if not DEBUG_OUTPUTS:
            return
        t = nc.dram_tensor("dbg_" + name, list(ap.shape), ap.dtype,
                           kind="ExternalOutput").ap()
        nc.sync.dma_start(t, ap)

    def din(name, shape, dt=F32R):
        return nc.dram_tensor(name, shape, dt, kind="ExternalInput").ap()

    zrT = din("zrT", [128, B])          # z_re transposed [n, b]
    ziT = din("ziT", [128, B])
    drT = din("drT", [128, B])          # d (unnormalized) transposed
    diT = din("diT", [128, B])
    zjrT = din("zjrT", [128, MLOC])     # codebook slice, transposed [n, m]
    zjiT = din("zjiT", [128, MLOC])
    djrT = din("djrT", [128, MLOC])
    djiT = din("djiT", [128, MLOC])
    Tcat = din("Tcat", [128, MT * 512])  # [T_re|T_im] m-chunked: col mc*512
    alpha_d = din("alpha", [128, MT], F32)   # per-m vectors, [m%128, m//128]
    sp_d = din("sigp", [128, MT], F32)
    sq_d = din("sigq", [128, MT], F32)

    out_d = nc.dram_tensor("out", [128, 512], F32, kind="ExternalOutput").ap()

    with tile.TileContext(nc) as tc:
        with tc.tile_pool(name="inp", bufs=1) as inp, \
             tc.tile_pool(name="dram", bufs=1, space="DRAM") as dram:
            # ---- persistent SBUF inputs (codebook first: pass1 needs it) ----
            djr = inp.tile([128, MLOC], F32R, tag="djr")
            dji = inp.tile([128, MLOC], F32R, tag="dji")
            zjr = inp.tile([128, MLOC], F32R, tag="zjr")
            zji = inp.tile([128, MLOC], F32R, tag="zji")
            zr = inp.tile([128, B], F32R, tag="zr")
            zi = inp.tile([128, B], F32R, tag="zi")
            dr = inp.tile([128, B], F32R, tag="dr")
            di = inp.tile([128, B], F32R, tag="di")
            al_sb = inp.tile([128, MT], F32, tag="al")
            sp_sb = inp.tile([128, MT], F32, tag="sp")
            sq_sb = inp.tile([128, MT], F32, tag="sq")
            for t, d in [(djr, djrT), (dji, djiT), (zjr, zjrT), (zji, zjiT),
                         (zr, zrT), (zi, ziT), (dr, drT), (di, diT),
                         (al_sb, alpha_d), (sp_sb, sp_d), (sq_sb, sq_d)]:
                nc.sync.dma_start(t[:], d)

            zr_v, zi_v = zr[:].bitcast(F32), zi[:].bitcast(F32)
            dr_v, di_v = dr[:].bitcast(F32), di[:].bitcast(F32)
            zjr_v, zji_v = zjr[:].bitcast(F32), zji[:].bitcast(F32)
            djr_v, dji_v = djr[:].bitcast(F32), dji[:].bitcast(F32)

            # derived full-width operands
            m2zjr = inp.tile([128, MLOC], F32R, tag="m2zjr")
            m2zji = inp.tile([128, MLOC], F32R, tag="m2zji")
            nji = inp.tile([128, MLOC], F32R, tag="nji")
            nc.vector.tensor_scalar_mul(m2zjr[:], zjr_v, -2.0)
            nc.vector.tensor_scalar_mul(m2zji[:], zji_v, -2.0)
            nc.vector.tensor_scalar_mul(nji[:], dji_v, -1.0)

            # ones / -ones helpers
            onef = inp.tile([128, 1], F32, tag="onef")
            monef = inp.tile([128, 1], F32, tag="monef")
            ones_c = inp.tile([128, 1], F32R, tag="ones_c")
            ones_b = inp.tile([128, 1], BF16, tag="ones_b")
            ones_r = inp.tile([1, 128], F32R, tag="ones_r")
            onef_r = inp.tile([1, 128], F32, tag="onef_r")
            ones_rB = inp.tile([1, BF], F32R, tag="ones_rB")
            onef_rB = inp.tile([1, BF], F32, tag="onef_rB")
            nc.vector.memset(onef[:], 1.0)
            nc.vector.memset(monef[:], -1.0)
            nc.vector.tensor_copy(ones_c[:], onef[:])
            nc.vector.tensor_copy(ones_b[:], onef[:])
            nc.vector.memset(onef_r[:], 1.0)
            nc.vector.tensor_copy(ones_r[:], onef_r[:])
            nc.vector.memset(onef_rB[:], 1.0)
            nc.vector.tensor_copy(ones_rB[:], onef_rB[:])

            # ---- preamble reductions over the partition (n) axis ----
            zn2row = inp.tile([1, B], F32R, tag="zn2row")
            zjn2row = inp.tile([1, MLOC], F32R, tag="zjn2row")
            dn2row = inp.tile([1, B], F32, tag="dn2row")
            djn2_sb = inp.tile([128, MT], F32, tag="djn2")
            mcpr_sb = inp.tile([128, MT], F32, tag="mcpr")
            mcpi_sb = inp.tile([128, MT], F32, tag="mcpi")

            with tc.tile_pool(name="prew", bufs=2) as prew, \
                 tc.tile_pool(name="prer", bufs=2) as prer, \
                 tc.tile_pool(name="preps", bufs=2, space="PSUM") as preps, \
                 tc.tile_pool(name="pmps", bufs=2, space="PSUM") as pmps:

                def products(pa, pb, sub=False):
                    """per-element pa[0]*pa[1] (+/-) pb[0]*pb[1] -> f32r tile"""
                    w = pa[0].shape[1]
                    u1 = prew.tile([128, w], F32, tag="u1", name="u1")
                    u2 = prew.tile([128, w], F32, tag="u2", name="u2")
                    u3 = prer.tile([128, w], F32R, tag="u3", name="u3")
                    nc.vector.tensor_mul(u1[:], pa[0], pa[1])
                    nc.vector.tensor_mul(u2[:], pb[0], pb[1])
                    nc.vector.tensor_tensor(u3[:], u1[:], u2[:],
                                            OP.subtract if sub else OP.add)
                    return u3

                def rowreduce(u3, dst):
                    """sum over partitions -> [1, W] row (via ones matmul)"""
                    w = dst[:].shape[1]
                    ps = preps.tile([1, w], F32, tag="ps", name="ps")
                    for h in range(w // 512):
                        sl = slice(h * 512, (h + 1) * 512)
                        nc.tensor.matmul(ps[0:1, sl], ones_c[:], u3[:, sl],
                                         start=True, stop=True)
                    nc.scalar.copy(dst[:], ps[:])

                def pmreduce(u3, dst, ones):
                    """sum over partitions -> per-partition [128, MT] layout
                    via per-m-tile matmul transpose (out[m,1] = u3_mt.T @ 1).
                    Plain fp32 (free dim 1 violates fp32r ISA restrictions)."""
                    for mt in range(MT):
                        msl = slice(mt * 128, (mt + 1) * 128)
                        ps1 = pmps.tile([128, 1], F32, tag="ps1", name="ps1")
                        nc.tensor.matmul(ps1[:], u3[:, msl].bitcast(F32),
                                         ones[:], start=True, stop=True)
                        nc.scalar.copy(dst[:, mt:mt + 1], ps1[:])

                u = products((zr_v, zr_v), (zi_v, zi_v))
                rowreduce(u, zn2row)
                u = products((zjr_v, zjr_v), (zji_v, zji_v))
                rowreduce(u, zjn2row)
                u = products((dr_v, dr_v), (di_v, di_v))
                rowreduce(u, dn2row)
                u = products((djr_v, djr_v), (dji_v, dji_v))
                pmreduce(u, djn2_sb, onef)
                u = products((djr_v, zjr_v), (dji_v, zji_v))
                pmreduce(u, mcpr_sb, monef)            # -> -cpr
                u = products((djr_v, zji_v), (dji_v, zjr_v), sub=True)
                pmreduce(u, mcpi_sb, monef)            # -> -cpi

            # ---- per-m scalar constants [128, MT] ----
            # c1 = pi/(2*sp*|dj|^2): s1 = sqrt(c1) (via exp/ln), b1 = -s1*cpr
            # k1n = -2*sp/pi, k2n = -sp/pi, c3 = -pi/sq, logg = ln gain
            s1_sb = inp.tile([128, MT], F32, tag="s1")
            s2_sb = inp.tile([128, MT], F32, tag="s2")
            b1_sb = inp.tile([128, MT], F32, tag="b1")
            b2_sb = inp.tile([128, MT], F32, tag="b2")
            k1n_sb = inp.tile([128, MT], F32, tag="k1n")
            k2n_sb = inp.tile([128, MT], F32, tag="k2n")
            c3_sb = inp.tile([128, MT], F32, tag="c3")
            logg_sb = inp.tile([128, MT], F32, tag="logg")
            t_a = inp.tile([128, MT], F32, tag="t_a")
            t_b = inp.tile([128, MT], F32, tag="t_b")
            t_c = inp.tile([128, MT], F32, tag="t_c")
            la = inp.tile([128, MT], F32, tag="la")
            lsp = inp.tile([128, MT], F32, tag="lsp")
            lr = inp.tile([128, MT], F32, tag="lr")

            nc.vector.tensor_mul(t_a[:], sp_sb[:], djn2_sb[:])
            nc.vector.reciprocal(t_b[:], t_a[:])       # 1/(sp*|dj|^2)
            nc.vector.tensor_scalar_mul(t_c[:], t_b[:], PI / 2.0)  # -c1
            nc.scalar.activation(lr[:], t_c[:], AF.Ln)
            nc.scalar.activation(s1_sb[:], lr[:], AF.Exp, scale=0.5)
            nc.vector.tensor_scalar_mul(s2_sb[:], s1_sb[:], math.sqrt(2.0))
            nc.vector.tensor_mul(b1_sb[:], s1_sb[:], mcpr_sb[:])
            nc.vector.tensor_mul(b2_sb[:], s2_sb[:], mcpi_sb[:])
            nc.vector.tensor_scalar_mul(k1n_sb[:], sp_sb[:], -2.0 / PI)
            nc.vector.tensor_scalar_mul(k2n_sb[:], sp_sb[:], -1.0 / PI)
            nc.vector.reciprocal(t_a[:], sq_sb[:])
            nc.vector.tensor_scalar_mul(c3_sb[:], t_a[:], -PI)
            nc.scalar.activation(la[:], al_sb[:], AF.Ln)
            nc.scalar.activation(lsp[:], sp_sb[:], AF.Ln)
            nc.scalar.activation(t_b[:], djn2_sb[:], AF.Ln)
            nc.vector.scalar_tensor_tensor(logg_sb[:], lsp[:], 0.5, la[:],
                                           op0=OP.mult, op1=OP.add)
            nc.vector.tensor_sub(logg_sb[:], logg_sb[:], t_b[:])
            nc.vector.tensor_scalar_add(logg_sb[:], logg_sb[:],
                                        -0.5 * math.log(2.0 * PI))
            dbg_dump("s1", s1_sb[:])
            dbg_dump("b1", b1_sb[:])
            dbg_dump("logg", logg_sb[:])
            dbg_dump("djn2", djn2_sb[:])

            # payload: load f32r, cast to bf16 (pass2 runs in bf16)
            tt32 = inp.tile([128, MT * 512], F32R, tag="tt32")
            tt16 = inp.tile([128, MT * 512], BF16, tag="tt16")
            nc.sync.dma_start(tt32[:], Tcat)
            for h in range(MT):
                sl = slice(h * 512, (h + 1) * 512)
                nc.vector.tensor_copy(tt16[:, sl], tt32[:, sl].bitcast(F32))

            # ---- main passes ----
            w_all = inp.tile([128, MT * B], BF16, tag="w_all")  # w'[m, b]
            den_cat = inp.tile([1, B], F32, tag="den_cat")      # b-permuted
            dn2p = inp.tile([1, B], F32, tag="dn2p")
            den_cat2 = inp.tile([1, B], F32, tag="den_cat2")

            num_bounce = dram.tile([B, 512], F32)
            den_bounce = dram.tile([1, B], F32)
            numsA = dram.tile([64, 512], F32)
            numsB = dram.tile([64, 512], F32)
            den_slice = dram.tile([1, 128], F32)

            # b-permuted view: flat pos c*128 + h*64 + j  <->  b = 512h+64c+j
            def pv(t, h):
                v = t[0:1, :].rearrange("o (c h j) -> o h c j",
                                        c=8, h=2, j=64)
                return v[:, h:h + 1].squeeze(1)

            def half_cj(ap):
                return ap.rearrange("o (c j) -> o c j", c=8, j=64)

            with tc.tile_pool(name="wk", bufs=5, space="PSUM") as wk, \
                 tc.tile_pool(name="ops", bufs=2, space="PSUM") as ops, \
                 tc.tile_pool(name="sb1", bufs=2) as sb1:

                def pass2(bc_list):
                    for bc in bc_list:
                        o_ps = ops.tile([128, 512], F32, tag="o_ps",
                                        name="o_ps", bufs=2)
                        for mc in range(MT):
                            lhs = w_all[:, mc * B + bc * 128:
                                        mc * B + bc * 128 + 128]
                            rhs = tt16[:, mc * 512:(mc + 1) * 512]
                            nc.tensor.matmul(o_ps[:], lhs, rhs,
                                             start=(mc == 0), stop=(mc == MT - 1))
                        o_cp = sb1.tile([128, 512], F32, tag="o_cp")
                        nc.scalar.copy(o_cp[:], o_ps[:])
                        nc.sync.dma_start(
                            num_bounce[bc * 128:(bc + 1) * 128, :], o_cp[:])

                for bh in range(BH):
                    bsl = slice(bh * BF, (bh + 1) * BF)
                    den_ps = ops.tile([1, BF], F32, tag="den_ps",
                                      name="den_ps", bufs=1)
                    for mt in range(MT):
                        msl = slice(mt * 128, (mt + 1) * 128)
                        ms1 = s1_sb[:, mt:mt + 1]
                        ms2 = s2_sb[:, mt:mt + 1]
                        mb1 = b1_sb[:, mt:mt + 1]
                        mb2 = b2_sb[:, mt:mt + 1]
                        mk1 = k1n_sb[:, mt:mt + 1]
                        mk2 = k2n_sb[:, mt:mt + 1]
                        mc3 = c3_sb[:, mt:mt + 1]
                        mlg = logg_sb[:, mt:mt + 1]

                        pr_ps = wk.tile([128, BF], F32, tag="wk", name="pr_ps")
                        pi_ps = wk.tile([128, BF], F32, tag="wk", name="pi_ps")
                        dot_ps = wk.tile([128, BF], F32, tag="wk", name="dot_ps")
                        ar_ps = wk.tile([128, BF], F32, tag="wk", name="ar_ps")
                        ai_ps = wk.tile([128, BF], F32, tag="wk", name="ai_ps")

                        nc.tensor.matmul(pr_ps[:], djr[:, msl], zr[:, bsl],
                                         start=True, stop=False)
                        nc.tensor.matmul(pr_ps[:], dji[:, msl], zi[:, bsl],
                                         start=False, stop=True)
                        nc.tensor.matmul(pi_ps[:], djr[:, msl], zi[:, bsl],
                                         start=True, stop=False)
                        nc.tensor.matmul(pi_ps[:], nji[:, msl], zr[:, bsl],
                                         start=False, stop=True)
                        nc.tensor.matmul(dot_ps[:], m2zjr[:, msl], zr[:, bsl],
                                         start=True, stop=False)
                        nc.tensor.matmul(dot_ps[:], m2zji[:, msl], zi[:, bsl],
                                         start=False, stop=False)
                        # dz2 = zn2[b] + zjn2[m] - 2*dot via two rank-1 updates
                        nc.tensor.matmul(dot_ps[:], ones_r[:],
                                         zn2row[0:1, bsl],
                                         start=False, stop=False)
                        nc.tensor.matmul(dot_ps[:], zjn2row[0:1, msl],
                                         ones_rB[:],
                                         start=False, stop=True)
                        nc.tensor.matmul(ar_ps[:], djr[:, msl], dr[:, bsl],
                                         start=True, stop=False)
                        nc.tensor.matmul(ar_ps[:], dji[:, msl], di[:, bsl],
                                         start=False, stop=True)
                        nc.tensor.matmul(ai_ps[:], djr[:, msl], di[:, bsl],
                                         start=True, stop=False)
                        nc.tensor.matmul(ai_ps[:], nji[:, msl], dr[:, bsl],
                                         start=False, stop=True)

                        # c1p = c1*pr_u^2, c2p = c2*pi_u^2 (both positive)
                        c1p = sb1.tile([128, BF], F32, tag="c1p")
                        c2p = sb1.tile([128, BF], F32, tag="c2p")
                        ar2 = sb1.tile([128, BF], BF16, tag="ar2")
                        ai2 = sb1.tile([128, BF], BF16, tag="ai2")
                        nc.scalar.activation(c1p[:], pr_ps[:], AF.Square,
                                             bias=mb1, scale=ms1)
                        nc.scalar.activation(c2p[:], pi_ps[:], AF.Square,
                                             bias=mb2, scale=ms2)
                        nc.scalar.activation(ar2[:], ar_ps[:], AF.Square)
                        nc.scalar.activation(ai2[:], ai_ps[:], AF.Square)

                        s = sb1.tile([128, BF], F32, tag="s")
                        s2 = sb1.tile([128, BF], F32, tag="s2t")
                        t1 = sb1.tile([128, BF], F32, tag="t1")
                        cs = sb1.tile([128, BF], F32, tag="cs")
                        e2 = sb1.tile([128, BF], F32, tag="e2")
                        eg = sb1.tile([128, BF], BF16, tag="eg")
                        align = sb1.tile([128, BF], BF16, tag="align")
                        # s2 = dz2 - (pr^2 + pi^2)/|dj|^2
                        nc.vector.scalar_tensor_tensor(s[:], c1p[:], mk1,
                                                       dot_ps[:],
                                                       op0=OP.mult, op1=OP.add)
                        nc.vector.scalar_tensor_tensor(s2[:], c2p[:], mk2,
                                                       s[:],
                                                       op0=OP.mult, op1=OP.add)
                        # t1 = min(c3*s2, 0) = c3 * relu(s2)   (c3 < 0)
                        nc.vector.tensor_scalar(t1[:], s2[:], mc3, 0.0,
                                                op0=OP.mult, op1=OP.min)
                        nc.gpsimd.tensor_add(cs[:], c1p[:], c2p[:])
                        nc.vector.tensor_sub(e2[:], t1[:], cs[:])
                        nc.scalar.activation(eg[:], e2[:], AF.Exp, bias=mlg)
                        nc.vector.tensor_add(align[:], ar2[:], ai2[:])
                        wv = w_all[:, mt * B + bh * BF: mt * B + bh * BF + BF]
                        nc.vector.tensor_tensor(wv, eg[:], align[:], OP.mult)
                        # denominator row accumulates on the PE
                        nc.tensor.matmul(den_ps[:], ones_b[:], wv,
                                         start=(mt == 0), stop=(mt == MT - 1))
                        if bh == 0 and mt == 0:
                            dbg_dump("c1p", c1p[:])
                            dbg_dump("c2p", c2p[:])
                            dbg_dump("s2", s2[:])
                            dbg_dump("e2", e2[:])
                            dbg_dump("w0", wv)

                    # partial denominator for this b-half, b-permuted layout
                    nc.scalar.copy(pv(den_cat, bh), half_cj(den_ps[0:1, :]))
                    nc.scalar.copy(pv(dn2p, bh), half_cj(dn2row[0:1, bsl]))

                    pass2([0, 1, 2, 3] if bh == 0 else [4, 5, 6, 7])
                    if bh == 0:
                        # overlap: reduce-scatter first num half mid-kernel
                        nc.gpsimd.collective_compute(
                            "ReduceScatter", OP.add,
                            replica_groups=[list(range(NCORES))],
                            ins=[num_bounce[0:512, :].opt()],
                            outs=[numsA.opt()])

                # fold eps/8 * |d_b|^2 into partial den, ship to collectives
                nc.vector.scalar_tensor_tensor(den_cat2[:], dn2p[:],
                                               EPS_TOTAL / NCORES, den_cat[:],
                                               op0=OP.mult, op1=OP.add)
                nc.sync.dma_start(den_bounce[:], den_cat2[:])

                nc.gpsimd.collective_compute(
                    "ReduceScatter", OP.add,
                    replica_groups=[list(range(NCORES))],
                    ins=[num_bounce[512:1024, :].opt()], outs=[numsB.opt()])
                nc.gpsimd.collective_compute(
                    "ReduceScatter", OP.add,
                    replica_groups=[list(range(NCORES))],
                    ins=[den_bounce.opt()], outs=[den_slice.opt()])

            # ---- tail: divide own rows ----
            with tc.tile_pool(name="tps", bufs=1, space="PSUM") as tps, \
                 tc.tile_pool(name="tsb", bufs=1) as tsb:
                o_sb = tsb.tile([128, 512], F32, tag="o_sb")
                den_row = tsb.tile([1, 128], F32, tag="den_row")
                den_col = tsb.tile([128, 1], F32, tag="den_col")
                invden = tsb.tile([128, 1], F32, tag="invden")
                outf = tsb.tile([128, 512], F32, tag="outf")
                nc.sync.dma_start(o_sb[0:64, :], numsA[:])
                nc.sync.dma_start(o_sb[64:128, :], numsB[:])
                nc.sync.dma_start(den_row[:], den_slice[:])
                # transpose [1,128] -> [128,1] on the PE (fp32 rank-1 matmul)
                den_colps = tps.tile([128, 1], F32, tag="den_colps")
                nc.tensor.matmul(den_colps[:], den_row[:],
                                 onef_r[0:1, 0:1], start=True, stop=True)
                nc.scalar.copy(den_col[:], den_colps[:])
                nc.vector.reciprocal(invden[:], den_col[:])
                nc.vector.tensor_scalar_mul(outf[:], o_sb[:], invden[:])
                nc.sync.dma_start(out_d, outf[:])

    nc.compile()
    return nc


_NC_CACHE = None


def _get_nc():
    global _NC_CACHE
    if _NC_CACHE is None:
        _NC_CACHE = _build()
    return _NC_CACHE


def _in_maps(inputs):
    f32 = np.float32
    t = lambda a: np.ascontiguousarray(np.asarray(a, f32).T)
    zrT, ziT = t(inputs["z_re"]), t(inputs["z_im"])
    drT, diT = t(inputs["d_re"]), t(inputs["d_im"])
    permv = lambda v: np.ascontiguousarray(
        np.asarray(v, f32).reshape(MT, 128).T)
    maps = []
    for c in range(NCORES):
        sl = slice(c * MLOC, (c + 1) * MLOC)
        Tc = np.concatenate([np.asarray(inputs["T_re"][sl], f32),
                             np.asarray(inputs["T_im"][sl], f32)], axis=1)
        Tc = np.ascontiguousarray(
            Tc.reshape(MT, 128, 512).transpose(1, 0, 2).reshape(128, MT * 512))
        maps.append({
            "zrT": zrT, "ziT": ziT, "drT": drT, "diT": diT,
            "zjrT": t(inputs["zj_re"][sl]), "zjiT": t(inputs["zj_im"][sl]),
            "djrT": t(inputs["dj_re"][sl]), "djiT": t(inputs["dj_im"][sl]),
            "Tcat": Tc,
            "alpha": permv(inputs["alpha"][sl]),
            "sigp": permv(inputs["sigma_par"][sl]),
            "sigq": permv(inputs["sigma_perp"][sl]),
        })
    return maps


def _gather(results):
    # core c returns rows [64c, 64c+64) (first b-half) then
    # [512+64c, 512+64c+64) (second b-half)
    full = np.empty((B, 512), np.float32)
    for c in range(NCORES):
        o = results[c]["out"]
        full[64 * c:64 * c + 64] = o[0:64]
        full[512 + 64 * c:512 + 64 * c + 64] = o[64:128]
    return np.ascontiguousarray(
        np.stack([full[:, :S], full[:, S:]], axis=-1).astype(np.float32))


def run(inputs, trace=False):
    nc = _get_nc()
    res = run_bass_kernel_spmd(nc, _in_maps(inputs), list(range(NCORES)),
                               trace=trace)
    return _gather(res.results), res


def kernel(**inputs) -> np.ndarray:
    out, _ = run(inputs, trace=False)
    return out
